# revision 29
# baseline (speedup 1.0000x reference)
"""Trainium2 Bass kernel for nn_NextGenerationHyperGNN (gnn_message_passing).

Computes, for node features x [96, 64] and two tiny MLPs:
  causal[i,j]  = sigmoid(MLP2(relu(MLP1(concat(x_i, x_j)))))        [96, 96]
  conf[i,j,k]  = sigmoid(V2 . relu(V1a x_i + V1b x_j + V1c x_k+c1)) [96, 96, 96]
with zeroed diagonals / non-distinct triplets.  (edge_index is unused by the
reference and therefore ignored here.)

Sharding: the leading triplet axis i is split across 8 NeuronCores (12 source
nodes per core); node features and the MLP weights are replicated.  Each core
computes conf[i_local, :, :] (as [k, i_local*96+j]) and causal[i_local, :].

Per-core device program (steady-state ~80us on HW, measured via a For_i loop
around the whole body; DVE/ACT elementwise-bound at ~62us each, PE ~45us):
  - tiny K=64 matmuls project x through W1/V1 (pi/pj/ta/tb/tc, h on partitions)
  - s[h,(ii,j)] = ta_ii + tb_j via one broadcast tensor_tensor per h-chunk (bf16)
  - per (k, h-chunk): ONE fused add+relu producing t[128,1152] bf16 —
    DVE tensor_scalar (op0=add with per-partition tcc column, op1=max 0;
    4x perf mode, ~430ns) for 72 k's, ACT activation(Relu, bias=tcc col,
    ~1.27us) for 24 k's — act_every=4 balances both engines at ~62us.
  - contraction over h on the PE with a sliding one-hot stationary matrix:
    B[128, 255] is zero except column 127 = V2-chunk; lhsT = B[:, 127-d:127-d+32]
    is a 32-column window whose only nonzero column is at position d = k mod 32,
    so the matmul adds v2.t_k into PSUM row k of the 32-row block at base
    partition (k//32)*32 (bases 0/32/64 are the only legal matmul output
    partitions). The three 32-row blocks are distinct PE col-groups, and the k
    loop rotates blocks (0,32,64,1,33,...) so consecutive matmuls column-tile
    and overlap in the array (~2x PE win). Each (k,chunk) issues 3 matmuls
    (bank-aligned free segments 512/512/128); all 576 accumulate into one
    [128,1536] PSUM tile = all conf logits for this core. Each block's first
    matmul uses start=True (the has_written clear is col-group-scoped).
  - sigmoid (+c2) on ACT, distinct-triplet mask multiply on DVE, DMA out.
"""

import sys

sys.path.insert(0, "/opt/trn_rl_repo")

import numpy as np
import ml_dtypes

N, D, H = 96, 64, 256
NCORES = 8
NI = N // NCORES          # 12 source nodes per core
JI = NI * N               # 1152 (ii, j) pairs per core
SEGS = ((0, 512), (512, 512), (1024, 128))  # PSUM-bank-aligned free segments

_cache = {}


def build_nc(b3, c2, repeat=1, act_every=4, t_bufs=20, loop_n=1, loop_all=1,
             n_act=None, skip_mm=0, skip_tgen=0, m32=1, interleave=1):
    """Build + compile the per-core Bass program (SPMD: same program, per-core data).

    repeat: unrolled python-level repeats of the conf stage (output unchanged).
    loop_n: hardware For_i trip count around the conf stage (for timing).
    """
    import concourse.bacc as bacc
    import concourse.mybir as mybir
    from concourse import tile

    dt = mybir.dt
    f32, bf16 = dt.float32, dt.bfloat16
    AF = mybir.ActivationFunctionType
    OP = mybir.AluOpType

    nc = bacc.Bacc(None, target_bir_lowering=False)

    XT = nc.dram_tensor("XT", [D, N], f32, kind="ExternalInput")
    XTL = nc.dram_tensor("XTL", [D, NI], f32, kind="ExternalInput")
    W1 = nc.dram_tensor("W1", [D, 2, H], f32, kind="ExternalInput")
    V1 = nc.dram_tensor("V1", [D, 3 * H], f32, kind="ExternalInput")
    W2R = nc.dram_tensor("W2R", [128, 2, 128], bf16, kind="ExternalInput")
    W3 = nc.dram_tensor("W3", [128, 1], bf16, kind="ExternalInput")
    V2R = nc.dram_tensor("V2R", [128, 2], bf16, kind="ExternalInput")
    B1 = nc.dram_tensor("B1", [128, 2], f32, kind="ExternalInput")
    B2 = nc.dram_tensor("B2", [128, 1], f32, kind="ExternalInput")
    C1 = nc.dram_tensor("C1", [128, 2], f32, kind="ExternalInput")
    MCONF = nc.dram_tensor("MCONF", [N, JI], f32, kind="ExternalInput")
    MCAUS = nc.dram_tensor("MCAUS", [1, JI], f32, kind="ExternalInput")
    B3C2 = nc.dram_tensor("B3C2", [128, 2], f32, kind="ExternalInput")
    CONF = nc.dram_tensor("CONF", [N, JI], f32, kind="ExternalOutput")
    CAUS = nc.dram_tensor("CAUS", [1, JI], f32, kind="ExternalOutput")

    with tile.TileContext(nc) as tc:
        with (
            tc.tile_pool(name="const", bufs=1) as cpool,
            tc.tile_pool(name="work", bufs=2) as wpool,
            tc.tile_pool(name="tp", bufs=t_bufs) as tpool,
            tc.tile_pool(name="psP", bufs=2, space="PSUM") as psP,
            tc.tile_pool(name="psA", bufs=1, space="PSUM") as psA,
            tc.tile_pool(name="psC", bufs=1, space="PSUM") as psC,
            tc.tile_pool(name="outp", bufs=1) as opool,
        ):
            xt = cpool.tile([D, N], f32)
            xtl = cpool.tile([D, NI], f32)
            w1 = cpool.tile([D, 2, H], f32)
            v1 = cpool.tile([D, 3 * H], f32)
            w2b = cpool.tile([128, 2, 128], bf16)
            w3b = cpool.tile([128, 1], bf16)
            v2 = cpool.tile([128, 2], bf16)
            b1 = cpool.tile([128, 2], f32)
            b2 = cpool.tile([128, 1], f32)
            c1 = cpool.tile([128, 2], f32)
            mconf = cpool.tile([N, JI], f32)
            mcaus = cpool.tile([1, JI], f32)
            b3c2 = cpool.tile([128, 2], f32)
            for sb, dr in ((xt, XT), (xtl, XTL), (w1, W1), (v1, V1), (w2b, W2R),
                           (w3b, W3), (v2, V2R), (b1, B1), (b2, B2), (c1, C1),
                           (mconf, MCONF), (mcaus, MCAUS), (b3c2, B3C2)):
                nc.sync.dma_start(sb[:], dr[:])

            # persistent tiles, written once per pass
            ta_sb = cpool.tile([128, 2, NI], f32)
            tb_sb = cpool.tile([128, 2, N], f32)
            tcc = cpool.tile([128, 2, N], f32)     # tcT + c1
            pi_sb = cpool.tile([128, 2, NI], f32)
            pj_sb = cpool.tile([128, 2, N], f32)
            s = cpool.tile([128, 2, JI], bf16)
            h1 = cpool.tile([128, 2, JI], bf16)
            caus_sb = opool.tile([1, JI], f32)
            conf_sb = opool.tile([N, JI], f32)
            psum_conf = psC.tile([128, 1536], f32)

            # sliding one-hot stationary buffers (init once)
            bmat = cpool.tile([128, 2, 255], bf16)
            nc.vector.memset(bmat[:], 0.0)
            for c in range(2):
                nc.vector.tensor_copy(bmat[:, c, 127:128], v2[:, c:c + 1])

            if n_act is None:
                act_ks = set(k for k in range(N) if k % act_every == act_every - 1)
            else:
                act_ks = set(k for k in range(N) if (k * n_act) % N < n_act)
            if interleave:
                k_order = [b + 32 * g for b in range(32) for g in range(3)]
            else:
                k_order = list(range(N))
            k_first, k_last = k_order[0], k_order[-1]

            t_fixed = (tpool.tile([128, JI], bf16, tag="tfix", name="t_fixed")
                       if (skip_tgen or skip_mm) else None)

            def stage_a():
                # projections (h-chunk c on partitions), then s = ta (+) tb in bf16
                for c in range(2):
                    for dst, lhsT, rhs in (
                        (ta_sb, v1[:, 128 * c:128 * c + 128], xtl),
                        (tb_sb, v1[:, H + 128 * c:H + 128 * c + 128], xt),
                        (pi_sb, w1[:, 0, 128 * c:128 * c + 128], xtl),
                        (pj_sb, w1[:, 1, 128 * c:128 * c + 128], xt),
                    ):
                        m = dst.shape[2]
                        ps = psP.tile([128, N], f32, tag="proj", name="ps")
                        nc.tensor.matmul(ps[:, :m], lhsT=lhsT, rhs=rhs[:], start=True, stop=True)
                        nc.vector.tensor_copy(dst[:, c, :], ps[:, :m])
                    ps = psP.tile([128, N], f32, tag="proj", name="ps")
                    nc.tensor.matmul(ps[:], lhsT=v1[:, 2 * H + 128 * c:2 * H + 128 * c + 128],
                                     rhs=xt[:], start=True, stop=True)
                    nc.scalar.activation(tcc[:, c, :], ps[:], AF.Identity, bias=c1[:, c:c + 1])
                for c in range(2):
                    nc.vector.tensor_tensor(
                        out=s[:, c, :].rearrange("p (a b) -> p a b", a=NI),
                        in0=ta_sb[:, c, :].unsqueeze(2).broadcast_to([128, NI, N]),
                        in1=tb_sb[:, c, :].unsqueeze(1).broadcast_to([128, NI, N]),
                        op=OP.add)

            def causal_head():
                for c in range(2):
                    h1r = wpool.tile([128, JI], f32, tag="h1r", name="h1r")
                    nc.vector.tensor_tensor(
                        out=h1r[:].rearrange("p (a b) -> p a b", a=NI),
                        in0=pi_sb[:, c, :].unsqueeze(2).broadcast_to([128, NI, N]),
                        in1=pj_sb[:, c, :].unsqueeze(1).broadcast_to([128, NI, N]),
                        op=OP.add)
                    nc.scalar.activation(h1[:, c, :], h1r[:], AF.Relu, bias=b1[:, c:c + 1])
                ps_h2 = psA.tile([128, 1536], f32, tag="caus", name="ps_h2")
                for c in range(2):
                    for off, sz in SEGS:
                        nc.tensor.matmul(ps_h2[:, off:off + sz], lhsT=w2b[:, c, :],
                                         rhs=h1[:, c, off:off + sz],
                                         start=(c == 0), stop=(c == 1))
                h2 = wpool.tile([128, JI], bf16, tag="h2", name="h2")
                nc.scalar.activation(h2[:], ps_h2[:, :JI], AF.Relu, bias=b2[:, 0:1])
                ps_cl = psA.tile([1, 1536], f32, tag="caus", name="ps_cl")
                for off, sz in SEGS:
                    nc.tensor.matmul(ps_cl[0:1, off:off + sz], lhsT=w3b[:, 0:1],
                                     rhs=h2[:, off:off + sz], start=True, stop=True)
                nc.scalar.activation(caus_sb[:], ps_cl[0:1, :JI], AF.Sigmoid,
                                     bias=b3c2[0:1, 0:1])
                nc.vector.tensor_tensor(out=caus_sb[:], in0=caus_sb[:], in1=mcaus[:], op=OP.mult)
                nc.sync.dma_start(CAUS[:], caus_sb[:])

            def conf_stage():
                if t_fixed is not None:
                    nc.vector.tensor_copy(t_fixed[:], s[:, 0, :])
                if skip_mm:  # keep PSUM initialized for the tail sigmoid
                    for off, sz in SEGS:
                        nc.tensor.matmul(psum_conf[:, off:off + sz], lhsT=bmat[:, 0, 127:255],
                                         rhs=t_fixed[:, off:off + sz], start=(off == 0),
                                         stop=(off == 1024), skip_group_check=True)
                for r in range(repeat):
                    for k in k_order:
                        use_act = k in act_ks
                        for c in range(2):
                            if skip_tgen:
                                t_kc = t_fixed
                            else:
                                t_kc = tpool.tile([128, JI], bf16, tag="t", name="t_kc")
                                if use_act:
                                    nc.scalar.activation(t_kc[:], s[:, c, :], AF.Relu,
                                                         bias=tcc[:, c, k:k + 1], scale=1.0)
                                else:
                                    nc.vector.tensor_scalar(out=t_kc[:], in0=s[:, c, :],
                                                            scalar1=tcc[:, c, k:k + 1],
                                                            scalar2=0.0, op0=OP.add, op1=OP.max)
                            if skip_mm:
                                continue
                            if m32:
                                base = (k // 32) * 32
                                lhsT = bmat[:, c, 127 - (k - base):127 - (k - base) + 32]
                                outsl = psum_conf[base:base + 32, :]
                                # start=True's has_written clear is scoped to the
                                # written col-group: each 32-row block needs its own
                                start = (k == base and c == 0)
                            else:
                                lhsT = bmat[:, c, 127 - k:255 - k]
                                outsl = psum_conf
                                start = (k == k_first and c == 0)
                            for off, sz in SEGS:
                                nc.tensor.matmul(outsl[:, off:off + sz],
                                                 lhsT=lhsT,
                                                 rhs=t_kc[:, off:off + sz],
                                                 start=start,
                                                 stop=(k == k_last and c == 1),
                                                 skip_group_check=True)

            def tail():
                # split by row halves so sigmoid/mask/DMA pipeline across engines
                for r0, r1 in ((0, N),):
                    nc.scalar.activation(conf_sb[r0:r1, :], psum_conf[r0:r1, :JI],
                                         AF.Sigmoid, bias=b3c2[r0:r1, 1:2])
                    nc.vector.tensor_tensor(out=conf_sb[r0:r1, :], in0=conf_sb[r0:r1, :],
                                            in1=mconf[r0:r1, :], op=OP.mult)
                    nc.sync.dma_start(CONF[r0:r1, :], conf_sb[r0:r1, :])

            def full_body():
                stage_a()
                causal_head()
                conf_stage()
                tail()

            if loop_all > 1:
                with tc.For_i(0, loop_all, 1, hint_engines=(mybir.EngineType.PE,)):
                    full_body()
                pass
            elif loop_n > 1:
                stage_a()
                causal_head()
                with tc.For_i(0, loop_n, 1, hint_engines=(mybir.EngineType.PE,)):
                    conf_stage()
                tail()
            else:
                full_body()

    nc.compile()
    return nc


def make_in_maps(inputs):
    """Per-core input dicts from the full problem inputs."""
    f32 = np.float32
    bf16 = ml_dtypes.bfloat16
    x = np.asarray(inputs["node_features"], f32)
    W1 = np.asarray(inputs["W1"], f32)
    b1 = np.asarray(inputs["b1"], f32)
    W2 = np.asarray(inputs["W2"], f32)
    b2 = np.asarray(inputs["b2"], f32)
    W3 = np.asarray(inputs["W3"], f32)
    V1 = np.asarray(inputs["V1"], f32)
    c1 = np.asarray(inputs["c1"], f32)
    V2 = np.asarray(inputs["V2"], f32)

    xT = np.ascontiguousarray(x.T)
    W1s = np.ascontiguousarray(W1.reshape(2, D, H).transpose(1, 0, 2))
    V1cat = np.ascontiguousarray(np.concatenate([V1[0:D], V1[D:2 * D], V1[2 * D:]], axis=1))
    W2r = np.ascontiguousarray(W2.reshape(2, 128, 128).transpose(1, 0, 2)).astype(bf16)
    W3b = np.ascontiguousarray(W3).astype(bf16)
    V2r = np.ascontiguousarray(V2.reshape(2, 128).T).astype(bf16)
    B1v = np.ascontiguousarray(b1.reshape(2, 128).T)
    B2v = np.ascontiguousarray(b2.reshape(128, 1))
    C1v = np.ascontiguousarray(c1.reshape(2, 128).T)

    b3 = np.float32(np.asarray(inputs["b3"]).reshape(-1)[0])
    c2 = np.float32(np.asarray(inputs["c2"]).reshape(-1)[0])
    b3c2 = np.ascontiguousarray(np.broadcast_to(np.array([b3, c2], f32), (128, 2)))

    jj = np.arange(N)
    kk = np.arange(N)
    in_maps = []
    for cidx in range(NCORES):
        ii = cidx * NI + np.arange(NI)
        mconf = ((ii[None, :, None] != jj[None, None, :])
                 & (jj[None, None, :] != kk[:, None, None])
                 & (ii[None, :, None] != kk[:, None, None]))
        mconf = mconf.astype(f32).reshape(N, JI)
        mcaus = (ii[:, None] != jj[None, :]).astype(f32).reshape(1, JI)
        in_maps.append({
            "XT": xT, "XTL": np.ascontiguousarray(xT[:, ii]),
            "W1": W1s, "V1": V1cat, "W2R": W2r, "W3": W3b, "V2R": V2r,
            "B1": B1v, "B2": B2v, "C1": C1v,
            "MCONF": np.ascontiguousarray(mconf), "MCAUS": mcaus, "B3C2": b3c2,
        })
    return in_maps


def make_runner(nc):
    """One-time jit of the SPMD NEFF executable; returns run(in_maps) -> per-core outputs.

    Mirrors concourse.bass2jax.run_bass_via_pjrt but builds the jitted callable
    once so repeated timed executions don't re-trace/re-compile.
    """
    import jax
    import concourse.mybir as mybir
    from concourse import bass2jax
    from jax.sharding import Mesh, PartitionSpec
    from jax.experimental.shard_map import shard_map

    bass2jax.install_neuronx_cc_hook()
    partition_name = nc.partition_id_tensor.name if nc.partition_id_tensor else None

    in_names, out_names, out_avals, zero_shapes = [], [], [], []
    for alloc in nc.m.functions[0].allocations:
        if not isinstance(alloc, mybir.MemoryLocationSet):
            continue
        name = alloc.memorylocations[0].name
        if alloc.kind == "ExternalInput":
            if name != partition_name:
                in_names.append(name)
        elif alloc.kind == "ExternalOutput":
            out_names.append(name)
            shape = tuple(alloc.tensor_shape)
            dtype = mybir.dt.np(alloc.dtype)
            out_avals.append(jax.core.ShapedArray(shape, dtype))
            zero_shapes.append((shape, dtype))
    n_params = len(in_names)
    all_names = list(in_names) + list(out_names)
    if partition_name is not None:
        all_names.append(partition_name)

    def _body(*args):
        operands = list(args)
        if partition_name is not None:
            operands.append(bass2jax.partition_id_tensor())
        outs = bass2jax._bass_exec_p.bind(
            *operands,
            out_avals=tuple(out_avals),
            in_names=tuple(all_names),
            out_names=tuple(out_names),
            lowering_input_output_aliases=(),
            sim_require_finite=True,
            sim_require_nnan=True,
            nc=nc,
        )
        return tuple(outs)

    donate = tuple(range(n_params, n_params + len(out_names)))
    devices = jax.devices()[:NCORES]
    mesh = Mesh(np.asarray(devices), ("core",))
    in_specs = (PartitionSpec("core"),) * (n_params + len(out_names))
    out_specs = (PartitionSpec("core"),) * len(out_names)
    sharded = jax.jit(
        shard_map(_body, mesh=mesh, in_specs=in_specs, out_specs=out_specs,
                  check_rep=False),
        donate_argnums=donate, keep_unused=True)

    def run(in_maps, device_arrays=False):
        concat_in = [np.concatenate([np.asarray(m[name]) for m in in_maps], axis=0)
                     for name in in_names]
        zeros = [np.zeros((NCORES * s[0], *s[1:]), d) for s, d in zero_shapes]
        out_arrs = sharded(*concat_in, *zeros)
        if device_arrays:
            return out_arrs
        return [
            {name: np.asarray(out_arrs[i]).reshape(NCORES, *zero_shapes[i][0])[c]
             for i, name in enumerate(out_names)}
            for c in range(NCORES)
        ]

    run.out_names = out_names
    return run


def assemble(results):
    causal = np.empty((N, N), np.float32)
    conf = np.empty((N, N, N), np.float32)
    for cidx in range(NCORES):
        A = results[cidx]["CONF"]                       # [k, ii*96+j]
        conf[cidx * NI:(cidx + 1) * NI] = A.reshape(N, NI, N).transpose(1, 2, 0)
        causal[cidx * NI:(cidx + 1) * NI] = results[cidx]["CAUS"][0].reshape(NI, N)
    return causal, conf


def _get_compiled(b3=0.0, c2=0.0, repeat=1, loop_n=1, **kw):
    key = (repeat, loop_n, tuple(sorted(kw.items())))
    if key not in _cache:
        nc = build_nc(0.0, 0.0, repeat=repeat, loop_n=loop_n, **kw)
        _cache[key] = (nc, make_runner(nc))
    return _cache[key]


def kernel(**inputs):
    from concourse.bass_utils import run_bass_kernel_spmd

    key = ("spmd_nc",)
    if key not in _cache:
        _cache[key] = build_nc(0.0, 0.0)
    nc = _cache[key]
    res = run_bass_kernel_spmd(nc, make_in_maps(inputs), core_ids=list(range(NCORES)))
    return assemble(res.results)


# revision 33
# speedup vs baseline: 1.0116x; 1.0116x over previous
"""Trainium2 Bass kernel for nn_NextGenerationHyperGNN (gnn_message_passing).

Computes, for node features x [96, 64] and two tiny MLPs:
  causal[i,j]  = sigmoid(MLP2(relu(MLP1(concat(x_i, x_j)))))        [96, 96]
  conf[i,j,k]  = sigmoid(V2 . relu(V1a x_i + V1b x_j + V1c x_k+c1)) [96, 96, 96]
with zeroed diagonals / non-distinct triplets.  (edge_index is unused by the
reference and therefore ignored here.)

Sharding: the leading triplet axis i is split across 8 NeuronCores (12 source
nodes per core); node features and the MLP weights are replicated.  Each core
computes conf[i_local, :, :] (as [k, i_local*96+j]) and causal[i_local, :].

Per-core device program (steady-state ~80us on HW, measured via a For_i loop
around the whole body; DVE/ACT elementwise-bound at ~62us each, PE ~45us):
  - tiny K=64 matmuls project x through W1/V1 (pi/pj/ta/tb/tc, h on partitions)
  - s[h,(ii,j)] = ta_ii + tb_j via one broadcast tensor_tensor per h-chunk (bf16)
  - per (k, h-chunk): ONE fused add+relu producing t[128,1152] bf16 —
    DVE tensor_scalar (op0=add with per-partition tcc column, op1=max 0;
    4x perf mode, ~430ns) for 72 k's, ACT activation(Relu, bias=tcc col,
    ~1.27us) for 24 k's — act_every=4 balances both engines at ~62us.
  - contraction over h on the PE with a sliding one-hot stationary matrix:
    B[128, 255] is zero except column 127 = V2-chunk; lhsT = B[:, 127-d:127-d+32]
    is a 32-column window whose only nonzero column is at position d = k mod 32,
    so the matmul adds v2.t_k into PSUM row k of the 32-row block at base
    partition (k//32)*32 (bases 0/32/64 are the only legal matmul output
    partitions). The three 32-row blocks are distinct PE col-groups, and the k
    loop rotates blocks (0,32,64,1,33,...) so consecutive matmuls column-tile
    and overlap in the array (~2x PE win). Each (k,chunk) issues 3 matmuls
    (bank-aligned free segments 512/512/128); all 576 accumulate into one
    [128,1536] PSUM tile = all conf logits for this core. Each block's first
    matmul uses start=True (the has_written clear is col-group-scoped).
  - sigmoid (+c2) on ACT, distinct-triplet mask multiply on DVE, DMA out.
"""

import sys

sys.path.insert(0, "/opt/trn_rl_repo")

import numpy as np
import ml_dtypes

N, D, H = 96, 64, 256
NCORES = 8
NI = N // NCORES          # 12 source nodes per core
JI = NI * N               # 1152 (ii, j) pairs per core
SEGS = ((0, 512), (512, 512), (1024, 128))  # PSUM-bank-aligned free segments

_cache = {}


def build_nc(b3, c2, repeat=1, act_every=4, t_bufs=20, loop_n=1, loop_all=1,
             n_act=None, skip_mm=0, skip_tgen=0, m32=1, interleave=1):
    """Build + compile the per-core Bass program (SPMD: same program, per-core data).

    repeat: unrolled python-level repeats of the conf stage (output unchanged).
    loop_n: hardware For_i trip count around the conf stage (for timing).
    """
    import concourse.bacc as bacc
    import concourse.mybir as mybir
    from concourse import tile

    dt = mybir.dt
    f32, bf16 = dt.float32, dt.bfloat16
    AF = mybir.ActivationFunctionType
    OP = mybir.AluOpType

    nc = bacc.Bacc(None, target_bir_lowering=False)

    XT = nc.dram_tensor("XT", [D, N], f32, kind="ExternalInput")
    XTL = nc.dram_tensor("XTL", [D, NI], f32, kind="ExternalInput")
    W1 = nc.dram_tensor("W1", [D, 2, H], f32, kind="ExternalInput")
    V1 = nc.dram_tensor("V1", [D, 3 * H], f32, kind="ExternalInput")
    W2R = nc.dram_tensor("W2R", [128, 2, 128], bf16, kind="ExternalInput")
    W3 = nc.dram_tensor("W3", [128, 1], bf16, kind="ExternalInput")
    V2R = nc.dram_tensor("V2R", [128, 2], bf16, kind="ExternalInput")
    B1 = nc.dram_tensor("B1", [128, 2], f32, kind="ExternalInput")
    B2 = nc.dram_tensor("B2", [128, 1], f32, kind="ExternalInput")
    C1 = nc.dram_tensor("C1", [128, 2], f32, kind="ExternalInput")
    MCONF = nc.dram_tensor("MCONF", [N, JI], f32, kind="ExternalInput")
    MCAUS = nc.dram_tensor("MCAUS", [1, JI], f32, kind="ExternalInput")
    B3C2 = nc.dram_tensor("B3C2", [128, 2], f32, kind="ExternalInput")
    CONF = nc.dram_tensor("CONF", [N, JI], f32, kind="ExternalOutput")
    CAUS = nc.dram_tensor("CAUS", [1, JI], f32, kind="ExternalOutput")

    with tile.TileContext(nc) as tc:
        with (
            tc.tile_pool(name="const", bufs=1) as cpool,
            tc.tile_pool(name="work", bufs=2) as wpool,
            tc.tile_pool(name="tp", bufs=t_bufs) as tpool,
            tc.tile_pool(name="psP", bufs=2, space="PSUM") as psP,
            tc.tile_pool(name="psA", bufs=1, space="PSUM") as psA,
            tc.tile_pool(name="psC", bufs=1, space="PSUM") as psC,
            tc.tile_pool(name="outp", bufs=1) as opool,
        ):
            xt = cpool.tile([D, N], f32)
            xtl = cpool.tile([D, NI], f32)
            w1 = cpool.tile([D, 2, H], f32)
            v1 = cpool.tile([D, 3 * H], f32)
            w2b = cpool.tile([128, 2, 128], bf16)
            w3b = cpool.tile([128, 1], bf16)
            v2 = cpool.tile([128, 2], bf16)
            b1 = cpool.tile([128, 2], f32)
            b2 = cpool.tile([128, 1], f32)
            c1 = cpool.tile([128, 2], f32)
            mconf = cpool.tile([N, JI], f32)
            mcaus = cpool.tile([1, JI], f32)
            b3c2 = cpool.tile([128, 2], f32)
            for sb, dr in ((xt, XT), (xtl, XTL), (w1, W1), (v1, V1), (w2b, W2R),
                           (w3b, W3), (v2, V2R), (b1, B1), (b2, B2), (c1, C1),
                           (mconf, MCONF), (mcaus, MCAUS), (b3c2, B3C2)):
                nc.sync.dma_start(sb[:], dr[:])

            # per-pass tiles, written once per pass
            ta_sb = cpool.tile([128, 2, NI], f32)
            tb_sb = cpool.tile([128, 2, N], f32)
            pi_sb = cpool.tile([128, 2, NI], f32)
            pj_sb = cpool.tile([128, 2, N], f32)
            h1 = cpool.tile([128, 2, JI], bf16)
            caus_sb = opool.tile([1, JI], f32)
            psum_conf = psC.tile([128, 1536], f32)

            # sliding one-hot stationary buffers (init once)
            bmat = cpool.tile([128, 2, 255], bf16)
            nc.vector.memset(bmat[:], 0.0)
            for c in range(2):
                nc.vector.tensor_copy(bmat[:, c, 127:128], v2[:, c:c + 1])

            if n_act is None:
                act_ks = set(k for k in range(N) if k % act_every == act_every - 1)
            else:
                act_ks = set(k for k in range(N) if (k * n_act) % N < n_act)
            if interleave == 2:
                # block 0 finishes early (pairs with alternating blocks 1/2),
                # so its tail evacuation overlaps the remaining matmuls
                k_order = []
                for b in range(32):
                    k_order.append(b)
                    k_order.append(32 * (1 + b % 2) + b // 2 + 16 * (b % 2))
                k_order += [32 + 16 + b // 2 if False else 0 for b in []]  # no-op
                done = set(k_order)
                k_order += [k for g in (1, 2) for k in range(32 * g, 32 * g + 32)
                            if k not in done]
            elif interleave:
                k_order = [b + 32 * g for b in range(32) for g in range(3)]
            else:
                k_order = list(range(N))
            k_first, k_last = k_order[0], k_order[-1]
            blk_first = {}
            for k in k_order:
                blk_first.setdefault((k // 32) * 32, k)

            t_fixed = (tpool.tile([128, JI], bf16, tag="tfix", name="t_fixed")
                       if (skip_tgen or skip_mm) else None)

            def stage_a(s, tcc):
                # projections (h-chunk c on partitions), then s = ta (+) tb in bf16
                for c in range(2):
                    for dst, lhsT, rhs in (
                        (ta_sb, v1[:, 128 * c:128 * c + 128], xtl),
                        (tb_sb, v1[:, H + 128 * c:H + 128 * c + 128], xt),
                        (pi_sb, w1[:, 0, 128 * c:128 * c + 128], xtl),
                        (pj_sb, w1[:, 1, 128 * c:128 * c + 128], xt),
                    ):
                        m = dst.shape[2]
                        ps = psP.tile([128, N], f32, tag="proj", name="ps")
                        nc.tensor.matmul(ps[:, :m], lhsT=lhsT, rhs=rhs[:], start=True, stop=True)
                        nc.vector.tensor_copy(dst[:, c, :], ps[:, :m])
                    ps = psP.tile([128, N], f32, tag="proj", name="ps")
                    nc.tensor.matmul(ps[:], lhsT=v1[:, 2 * H + 128 * c:2 * H + 128 * c + 128],
                                     rhs=xt[:], start=True, stop=True)
                    nc.scalar.activation(tcc[:, c, :], ps[:], AF.Identity, bias=c1[:, c:c + 1])
                for c in range(2):
                    nc.vector.tensor_tensor(
                        out=s[:, c, :].rearrange("p (a b) -> p a b", a=NI),
                        in0=ta_sb[:, c, :].unsqueeze(2).broadcast_to([128, NI, N]),
                        in1=tb_sb[:, c, :].unsqueeze(1).broadcast_to([128, NI, N]),
                        op=OP.add)

            def causal_head():
                for c in range(2):
                    h1r = wpool.tile([128, JI], f32, tag="h1r", name="h1r")
                    nc.vector.tensor_tensor(
                        out=h1r[:].rearrange("p (a b) -> p a b", a=NI),
                        in0=pi_sb[:, c, :].unsqueeze(2).broadcast_to([128, NI, N]),
                        in1=pj_sb[:, c, :].unsqueeze(1).broadcast_to([128, NI, N]),
                        op=OP.add)
                    nc.scalar.activation(h1[:, c, :], h1r[:], AF.Relu, bias=b1[:, c:c + 1])
                ps_h2 = psA.tile([128, 1536], f32, tag="caus", name="ps_h2")
                for c in range(2):
                    for off, sz in SEGS:
                        nc.tensor.matmul(ps_h2[:, off:off + sz], lhsT=w2b[:, c, :],
                                         rhs=h1[:, c, off:off + sz],
                                         start=(c == 0), stop=(c == 1))
                h2 = wpool.tile([128, JI], bf16, tag="h2", name="h2")
                nc.scalar.activation(h2[:], ps_h2[:, :JI], AF.Relu, bias=b2[:, 0:1])
                ps_cl = psA.tile([1, 1536], f32, tag="caus", name="ps_cl")
                for off, sz in SEGS:
                    nc.tensor.matmul(ps_cl[0:1, off:off + sz], lhsT=w3b[:, 0:1],
                                     rhs=h2[:, off:off + sz], start=True, stop=True)
                nc.scalar.activation(caus_sb[:], ps_cl[0:1, :JI], AF.Sigmoid,
                                     bias=b3c2[0:1, 0:1])
                nc.vector.tensor_tensor(out=caus_sb[:], in0=caus_sb[:], in1=mcaus[:], op=OP.mult)
                nc.sync.dma_start(CAUS[:], caus_sb[:])

            def conf_stage(s, tcc):
                if t_fixed is not None:
                    nc.vector.tensor_copy(t_fixed[:], s[:, 0, :])
                if skip_mm:  # keep PSUM initialized for the tail sigmoid
                    for off, sz in SEGS:
                        nc.tensor.matmul(psum_conf[:, off:off + sz], lhsT=bmat[:, 0, 127:255],
                                         rhs=t_fixed[:, off:off + sz], start=(off == 0),
                                         stop=(off == 1024), skip_group_check=True)
                for r in range(repeat):
                    for k in k_order:
                        use_act = k in act_ks
                        for c in range(2):
                            if skip_tgen:
                                t_kc = t_fixed
                            else:
                                t_kc = tpool.tile([128, JI], bf16, tag="t", name="t_kc")
                                if use_act:
                                    nc.scalar.activation(t_kc[:], s[:, c, :], AF.Relu,
                                                         bias=tcc[:, c, k:k + 1], scale=1.0)
                                else:
                                    nc.vector.tensor_scalar(out=t_kc[:], in0=s[:, c, :],
                                                            scalar1=tcc[:, c, k:k + 1],
                                                            scalar2=0.0, op0=OP.add, op1=OP.max)
                            if skip_mm:
                                continue
                            if m32:
                                base = (k // 32) * 32
                                lhsT = bmat[:, c, 127 - (k - base):127 - (k - base) + 32]
                                outsl = psum_conf[base:base + 32, :]
                                # start=True's has_written clear is scoped to the
                                # written col-group: each 32-row block needs its own
                                start = (k == blk_first[base] and c == 0)
                            else:
                                lhsT = bmat[:, c, 127 - k:255 - k]
                                outsl = psum_conf
                                start = (k == k_first and c == 0)
                            for off, sz in SEGS:
                                nc.tensor.matmul(outsl[:, off:off + sz],
                                                 lhsT=lhsT,
                                                 rhs=t_kc[:, off:off + sz],
                                                 start=start,
                                                 stop=(k == k_last and c == 1),
                                                 skip_group_check=True)

            def tail(conf_sb):
                blocks = ((0, 32), (32, 64), (64, N)) if interleave == 2 else ((0, N),)
                for r0, r1 in blocks:
                    nc.scalar.activation(conf_sb[r0:r1, :], psum_conf[r0:r1, :JI],
                                         AF.Sigmoid, bias=b3c2[r0:r1, 1:2])
                    nc.vector.tensor_tensor(out=conf_sb[r0:r1, :], in0=conf_sb[r0:r1, :],
                                            in1=mconf[r0:r1, :], op=OP.mult)
                    nc.sync.dma_start(CONF[r0:r1, :], conf_sb[r0:r1, :])

            def full_body():
                # double-buffered per-pass tiles so the next pass's stage A and
                # matmuls don't WAR-stall on the previous pass's consumers
                s = cpool.tile([128, 2, JI], bf16, tag="s", bufs=2, name="s")
                tcc = cpool.tile([128, 2, N], f32, tag="tcc", bufs=2, name="tcc")
                conf_sb = opool.tile([N, JI], f32, tag="conf_sb", bufs=2, name="conf_sb")
                stage_a(s, tcc)
                causal_head()
                conf_stage(s, tcc)
                tail(conf_sb)

            if loop_all > 1:
                with tc.For_i(0, loop_all, 1, hint_engines=(mybir.EngineType.PE,)):
                    full_body()
            elif loop_n > 1:
                s = cpool.tile([128, 2, JI], bf16, tag="s", bufs=2, name="s")
                tcc = cpool.tile([128, 2, N], f32, tag="tcc", bufs=2, name="tcc")
                conf_sb = opool.tile([N, JI], f32, tag="conf_sb", bufs=2, name="conf_sb")
                stage_a(s, tcc)
                causal_head()
                with tc.For_i(0, loop_n, 1, hint_engines=(mybir.EngineType.PE,)):
                    conf_stage(s, tcc)
                tail(conf_sb)
            else:
                full_body()

    nc.compile()
    return nc


def make_in_maps(inputs):
    """Per-core input dicts from the full problem inputs."""
    f32 = np.float32
    bf16 = ml_dtypes.bfloat16
    x = np.asarray(inputs["node_features"], f32)
    W1 = np.asarray(inputs["W1"], f32)
    b1 = np.asarray(inputs["b1"], f32)
    W2 = np.asarray(inputs["W2"], f32)
    b2 = np.asarray(inputs["b2"], f32)
    W3 = np.asarray(inputs["W3"], f32)
    V1 = np.asarray(inputs["V1"], f32)
    c1 = np.asarray(inputs["c1"], f32)
    V2 = np.asarray(inputs["V2"], f32)

    xT = np.ascontiguousarray(x.T)
    W1s = np.ascontiguousarray(W1.reshape(2, D, H).transpose(1, 0, 2))
    V1cat = np.ascontiguousarray(np.concatenate([V1[0:D], V1[D:2 * D], V1[2 * D:]], axis=1))
    W2r = np.ascontiguousarray(W2.reshape(2, 128, 128).transpose(1, 0, 2)).astype(bf16)
    W3b = np.ascontiguousarray(W3).astype(bf16)
    V2r = np.ascontiguousarray(V2.reshape(2, 128).T).astype(bf16)
    B1v = np.ascontiguousarray(b1.reshape(2, 128).T)
    B2v = np.ascontiguousarray(b2.reshape(128, 1))
    C1v = np.ascontiguousarray(c1.reshape(2, 128).T)

    b3 = np.float32(np.asarray(inputs["b3"]).reshape(-1)[0])
    c2 = np.float32(np.asarray(inputs["c2"]).reshape(-1)[0])
    b3c2 = np.ascontiguousarray(np.broadcast_to(np.array([b3, c2], f32), (128, 2)))

    jj = np.arange(N)
    kk = np.arange(N)
    in_maps = []
    for cidx in range(NCORES):
        ii = cidx * NI + np.arange(NI)
        mconf = ((ii[None, :, None] != jj[None, None, :])
                 & (jj[None, None, :] != kk[:, None, None])
                 & (ii[None, :, None] != kk[:, None, None]))
        mconf = mconf.astype(f32).reshape(N, JI)
        mcaus = (ii[:, None] != jj[None, :]).astype(f32).reshape(1, JI)
        in_maps.append({
            "XT": xT, "XTL": np.ascontiguousarray(xT[:, ii]),
            "W1": W1s, "V1": V1cat, "W2R": W2r, "W3": W3b, "V2R": V2r,
            "B1": B1v, "B2": B2v, "C1": C1v,
            "MCONF": np.ascontiguousarray(mconf), "MCAUS": mcaus, "B3C2": b3c2,
        })
    return in_maps


def make_runner(nc):
    """One-time jit of the SPMD NEFF executable; returns run(in_maps) -> per-core outputs.

    Mirrors concourse.bass2jax.run_bass_via_pjrt but builds the jitted callable
    once so repeated timed executions don't re-trace/re-compile.
    """
    import jax
    import concourse.mybir as mybir
    from concourse import bass2jax
    from jax.sharding import Mesh, PartitionSpec
    from jax.experimental.shard_map import shard_map

    bass2jax.install_neuronx_cc_hook()
    partition_name = nc.partition_id_tensor.name if nc.partition_id_tensor else None

    in_names, out_names, out_avals, zero_shapes = [], [], [], []
    for alloc in nc.m.functions[0].allocations:
        if not isinstance(alloc, mybir.MemoryLocationSet):
            continue
        name = alloc.memorylocations[0].name
        if alloc.kind == "ExternalInput":
            if name != partition_name:
                in_names.append(name)
        elif alloc.kind == "ExternalOutput":
            out_names.append(name)
            shape = tuple(alloc.tensor_shape)
            dtype = mybir.dt.np(alloc.dtype)
            out_avals.append(jax.core.ShapedArray(shape, dtype))
            zero_shapes.append((shape, dtype))
    n_params = len(in_names)
    all_names = list(in_names) + list(out_names)
    if partition_name is not None:
        all_names.append(partition_name)

    def _body(*args):
        operands = list(args)
        if partition_name is not None:
            operands.append(bass2jax.partition_id_tensor())
        outs = bass2jax._bass_exec_p.bind(
            *operands,
            out_avals=tuple(out_avals),
            in_names=tuple(all_names),
            out_names=tuple(out_names),
            lowering_input_output_aliases=(),
            sim_require_finite=True,
            sim_require_nnan=True,
            nc=nc,
        )
        return tuple(outs)

    donate = tuple(range(n_params, n_params + len(out_names)))
    devices = jax.devices()[:NCORES]
    mesh = Mesh(np.asarray(devices), ("core",))
    in_specs = (PartitionSpec("core"),) * (n_params + len(out_names))
    out_specs = (PartitionSpec("core"),) * len(out_names)
    sharded = jax.jit(
        shard_map(_body, mesh=mesh, in_specs=in_specs, out_specs=out_specs,
                  check_rep=False),
        donate_argnums=donate, keep_unused=True)

    def run(in_maps, device_arrays=False):
        concat_in = [np.concatenate([np.asarray(m[name]) for m in in_maps], axis=0)
                     for name in in_names]
        zeros = [np.zeros((NCORES * s[0], *s[1:]), d) for s, d in zero_shapes]
        out_arrs = sharded(*concat_in, *zeros)
        if device_arrays:
            return out_arrs
        return [
            {name: np.asarray(out_arrs[i]).reshape(NCORES, *zero_shapes[i][0])[c]
             for i, name in enumerate(out_names)}
            for c in range(NCORES)
        ]

    run.out_names = out_names
    return run


def assemble(results):
    causal = np.empty((N, N), np.float32)
    conf = np.empty((N, N, N), np.float32)
    for cidx in range(NCORES):
        A = results[cidx]["CONF"]                       # [k, ii*96+j]
        conf[cidx * NI:(cidx + 1) * NI] = A.reshape(N, NI, N).transpose(1, 2, 0)
        causal[cidx * NI:(cidx + 1) * NI] = results[cidx]["CAUS"][0].reshape(NI, N)
    return causal, conf


def _get_compiled(b3=0.0, c2=0.0, repeat=1, loop_n=1, **kw):
    key = (repeat, loop_n, tuple(sorted(kw.items())))
    if key not in _cache:
        nc = build_nc(0.0, 0.0, repeat=repeat, loop_n=loop_n, **kw)
        _cache[key] = (nc, make_runner(nc))
    return _cache[key]


def kernel(**inputs):
    from concourse.bass_utils import run_bass_kernel_spmd

    key = ("spmd_nc",)
    if key not in _cache:
        _cache[key] = build_nc(0.0, 0.0)
    nc = _cache[key]
    res = run_bass_kernel_spmd(nc, make_in_maps(inputs), core_ids=list(range(NCORES)))
    return assemble(res.results)
